# revision 19
# baseline (speedup 1.0000x reference)
"""MinDist (k=1 nearest neighbor within batch slab) Trainium2 kernel.

Problem: x [B=64, Nn=1024, d=128] f32, batch [1024] int64 (all zeros =>
one graph per slab).  Output = x - x[nn_idx] where nn_idx is the argmin
of masked pairwise squared distances per slab.

Strategy (data-parallel over B across 8 cores, 8 slabs per core):
  For each slab:
    NegD[i, j] = x_i . x_j - 0.5*||x_j||^2      (argmax_j NegD = argmin_j d2)
  The self column j == i is always the strict top-1 of each row (d2 == 0),
  so no diagonal mask is needed: take the SECOND entry of the DVE top-8
  (max / max_index) as the nearest neighbor.  Gather x[nn] with an
  indirect DMA and subtract.

Precision: the gram is computed as bf16 hi/lo split (hh + hl + lh three
matmuls, absolute error ~2e-4 on G — fp32-level), and the bias row is a
three-way bf16 split (h/m/l) applied via K=3 matmuls per 128-column block.

Engines:
  PE    : transposes, gram matmuls (bf16 x3) + bias matmuls.
  ACT   : psum->sbuf copies, squares (+row-sum accum), bf16 split rounds.
  DVE   : max (top-8 values) + max_index per 128-row block.
  GPSIMD: split residuals, index offset add, indirect gather, subtract.
"""

import numpy as np

import concourse.bass as bass
import concourse.mybir as mybir
import concourse.tile as tile
from concourse.masks import make_identity

P = 128          # partitions / row-block size
D = 128          # feature dim
NN = 1024        # nodes per slab
NSLAB = 8        # slabs (batch elements) per core
NCORES = 8
NBLK = NN // P   # 8 row blocks per slab

F32 = mybir.dt.float32
BF16 = mybir.dt.bfloat16
U32 = mybir.dt.uint32

ACTF = mybir.ActivationFunctionType
ALU = mybir.AluOpType


def build_nc(split_waits=True):
    nc = bass.Bass()
    x_in = nc.declare_dram_parameter("x", [NSLAB * NN, D], F32, isOutput=False)
    out_ext = nc.declare_dram_parameter("out", [NSLAB * NN, D], F32, isOutput=True)

    with tile.TileContext(nc) as tc:
        with (
            tc.tile_pool(name="const", bufs=1) as cpool,
            tc.tile_pool(name="xhl", bufs=2) as xhlpool,
            tc.tile_pool(name="row", bufs=2) as rowpool,
            tc.tile_pool(name="small", bufs=4) as small,
            tc.tile_pool(name="io", bufs=4) as io,
            tc.tile_pool(name="tp", bufs=1, space="PSUM") as tppool,
            tc.tile_pool(name="tps", bufs=1, space="PSUM") as tpspool,
            tc.tile_pool(name="negd", bufs=3, space="PSUM") as ndpool,
        ):
            ident = cpool.tile([P, P], F32)
            make_identity(nc, ident[:])
            ones12 = cpool.tile([12, P], BF16)    # K=12 lhsT for bias matmul
            nc.gpsimd.memset(ones12[:], 1.0)

            for s in range(NSLAB):
                base = s * NN

                # ---- transpose slab -> xT (fp32, psum-roundtrip), and sq ----
                xT = rowpool.tile([P, NN], F32, tag="xT")
                sqcol = small.tile([P, NBLK], F32, tag="sqcol")
                for b in range(NBLK):
                    xi = io.tile([P, D], F32, tag="xin")
                    nc.sync.dma_start(xi[:], x_in[base + b * P: base + (b + 1) * P, :])
                    tp = tppool.tile([P, P], F32)
                    nc.tensor.transpose(tp[:], xi[:], ident[:])
                    nc.scalar.activation(xT[:, b * P:(b + 1) * P], tp[:], ACTF.Copy)
                    # sq for rows of this block, as a column
                    sqsc = small.tile([P, D], F32, tag="sqscratch")
                    nc.scalar.activation(
                        sqsc[:], xi[:], ACTF.Square,
                        accum_out=sqcol[:, b:b + 1],
                    )

                # hi/lo bf16 split of xT
                xh = xhlpool.tile([P, NN], BF16, tag="xh")
                nc.scalar.activation(xh[:], xT[:], ACTF.Copy)
                xl = xhlpool.tile([P, NN], BF16, tag="xl")
                nc.gpsimd.tensor_tensor(xl[:], xT[:], xh[:], op=ALU.subtract)

                # ---- bias row: s3[3k+t] = t'th bf16 split of -0.5*sq(block k)
                tps = tpspool.tile([NBLK, P], F32)
                nc.tensor.transpose(tps[:], sqcol[:], ident[:])
                sqr = small.tile([NBLK, P], F32, tag="sqr")
                nc.scalar.activation(sqr[:], tps[:], ACTF.Copy, scale=-0.5)
                hml = []
                prev = sqr
                for t in range(3):
                    st = small.tile([NBLK, P], BF16, tag=f"split{t}")
                    nc.scalar.activation(st[:], prev[:], ACTF.Copy)
                    hml.append(st)
                    if t < 2:
                        r = small.tile([NBLK, P], F32, tag=f"resid{t}")
                        nc.vector.scalar_tensor_tensor(
                            r[:], prev[:], 0.0, st[:], op0=ALU.add, op1=ALU.subtract
                        )
                        prev = r

                # zero-padded bias rhs: rhs_b[t*4+k', jb*512+k'*128+n]
                #   = split_t(-0.5*sq)[block 4*jb+k', n]
                rhs_b = rowpool.tile([12, NN], BF16, tag="rhsb")
                nc.gpsimd.memset(rhs_b[:], 0.0)
                for jb in range(2):
                    for kq in range(4):
                        for t in range(3):
                            eng = nc.sync if (kq + t) % 2 == 0 else nc.scalar
                            eng.dma_start(
                                rhs_b[t * 4 + kq: t * 4 + kq + 1,
                                      jb * 512 + kq * P: jb * 512 + (kq + 1) * P],
                                hml[t][4 * jb + kq: 4 * jb + kq + 1, :],
                            )

                base_col = small.tile([P, 1], U32, tag="basecol")
                nc.gpsimd.memset(base_col[:], base)

                # ---- per row-block: NegD, top-8, gather, subtract ----
                for b in range(NBLK):
                    bsl = slice(b * P, (b + 1) * P)
                    negd = ndpool.tile([P, NN], F32)
                    sls = [slice(0, 512), slice(512, 1024)]
                    for jb in range(2):
                        nc.tensor.matmul(
                            negd[:, sls[jb]], lhsT=xh[:, bsl], rhs=xh[:, sls[jb]],
                            start=True, stop=False,
                        )
                    for jb in range(2):
                        nc.tensor.matmul(
                            negd[:, sls[jb]], lhsT=xh[:, bsl], rhs=xl[:, sls[jb]],
                            start=False, stop=False,
                        )
                    for jb in range(2):
                        nc.tensor.matmul(
                            negd[:, sls[jb]], lhsT=xl[:, bsl], rhs=xh[:, sls[jb]],
                            start=False, stop=False,
                        )
                    for jb in range(2):
                        nc.tensor.matmul(
                            negd[:, sls[jb]], lhsT=ones12[:], rhs=rhs_b[:, sls[jb]],
                            start=False, stop=True,
                        )

                    v8 = small.tile([P, 8], F32, tag="v8")
                    nc.vector.max(v8[:], negd[:])
                    i8 = small.tile([P, 8], U32, tag="i8")
                    nc.vector.max_index(i8[:], v8[:], negd[:])

                    # second-best = nearest neighbor (top-1 is always self)
                    idxg = small.tile([P, 1], U32, tag="idxg")
                    nc.gpsimd.tensor_tensor(
                        idxg[:], i8[:, 1:2], base_col[:], op=ALU.add
                    )

                    xnn = io.tile([P, D], F32, tag="xnn")
                    nc.gpsimd.indirect_dma_start(
                        out=xnn[:],
                        out_offset=None,
                        in_=x_in[:],
                        in_offset=bass.IndirectOffsetOnAxis(ap=idxg[:, :1], axis=0),
                    )

                    xi2 = io.tile([P, D], F32, tag="xi2")
                    nc.scalar.dma_start(xi2[:], x_in[base + b * P: base + (b + 1) * P, :])
                    o = io.tile([P, D], F32, tag="o")
                    nc.gpsimd.tensor_tensor(o[:], xi2[:], xnn[:], op=ALU.subtract)
                    nc.sync.dma_start(out_ext[base + b * P: base + (b + 1) * P, :], o[:])

    if split_waits:
        _split_excess_waits(nc)
    return nc


# walrus codegen wait-count limits per instruction struct
_WAIT_LIMITS = ((mybir.InstMatmult, 1), (mybir.InstDMACopy, 1))


def _split_excess_waits(nc):
    """walrus codegen allows limited sync waits per instruction (e.g. one on
    a Matmult S3_LW struct); peel extra waits onto preceding same-engine
    NoOps."""
    for fn in nc.m.functions:
        for bb in fn.blocks:
            out = []
            for ins in bb.instructions:
                si = ins.sync_info
                limit = 1
                if si is not None and len(si.on_wait) > limit:
                    for k, w in enumerate(si.on_wait[:-limit]):
                        nop = mybir.InstNoOp(
                            name=f"{ins.name}-w{k}",
                            engine=ins.engine,
                            bass_nofuse=True,
                        )
                        nop.sync_info = mybir.SyncInfo(on_wait=[w], on_update=[])
                        out.append(nop)
                    ins.sync_info = mybir.SyncInfo(
                        on_wait=list(si.on_wait[-limit:]), on_update=si.on_update
                    )
                out.append(ins)
            bb.instructions[:] = out


_NC_CACHE = None
LAST_EXEC_NS = None


def _get_nc():
    global _NC_CACHE
    if _NC_CACHE is None:
        _NC_CACHE = build_nc()
    return _NC_CACHE


def _numpy_fallback(x, batch):
    # General (non-uniform batch) path; mirrors the reference exactly.
    x = np.asarray(x, dtype=np.float32)
    batch = np.asarray(batch)
    B, Nn, d = x.shape
    sq = np.sum(x * x, axis=-1)
    out = np.empty_like(x)
    same = batch[:, None] == batch[None, :]
    np.fill_diagonal(same, False)
    big = np.finfo(np.float32).max
    for b in range(B):
        d2 = sq[b][:, None] + sq[b][None, :] - 2.0 * (x[b] @ x[b].T)
        d2 = np.where(same, d2, big)
        nn_idx = np.argmin(d2, axis=-1)
        out[b] = x[b] - x[b][nn_idx]
    return out


def _ensure_ntff_hook():
    """The agent image's antenv lacks axon_hooks; synthesize it and register
    the ctypes NTFF profile hook so run_bass_kernel_spmd(trace=True) works."""
    import sys
    import types

    if "antenv.axon_hooks" in sys.modules:
        return
    try:
        from trn_agent_boot.trn_boot import _ntff_profile_via_ctypes
        hook = _ntff_profile_via_ctypes("/opt/axon/libaxon_pjrt.so")
    except Exception:
        hook = None
    mod = types.ModuleType("antenv.axon_hooks")
    mod._hook = hook

    def set_axon_ntff_profile_hook(h):
        mod._hook = h

    def get_axon_ntff_profile_hook():
        return mod._hook

    mod.set_axon_ntff_profile_hook = set_axon_ntff_profile_hook
    mod.get_axon_ntff_profile_hook = get_axon_ntff_profile_hook
    sys.modules["antenv.axon_hooks"] = mod


def kernel(x, batch):
    global LAST_EXEC_NS
    x = np.ascontiguousarray(np.asarray(x, dtype=np.float32))
    batch = np.asarray(batch)
    assert x.shape == (NCORES * NSLAB, NN, D), x.shape

    if np.unique(batch).size > 1:
        # batch grouping other than "one graph per slab" — host fallback
        return _numpy_fallback(x, batch)

    from concourse.bass_utils import run_bass_kernel_spmd

    nc = _get_nc()
    in_maps = [
        {"x": x[c * NSLAB:(c + 1) * NSLAB].reshape(NSLAB * NN, D)}
        for c in range(NCORES)
    ]
    import os
    trace = os.environ.get("KERNEL_TRACE", "1") == "1"
    if trace:
        _ensure_ntff_hook()
    res = run_bass_kernel_spmd(
        nc, in_maps, core_ids=list(range(NCORES)), trace=trace,
        tmpdir=os.environ.get("KERNEL_TRACE_DIR") or None,
    )
    LAST_EXEC_NS = res.exec_time_ns
    outs = [res.results[c]["out"].reshape(NSLAB, NN, D) for c in range(NCORES)]
    return np.concatenate(outs, axis=0)


# revision 20
# speedup vs baseline: 1.2676x; 1.2676x over previous
"""MinDist (k=1 nearest neighbor within batch slab) Trainium2 kernel.

Problem: x [B=64, Nn=1024, d=128] f32, batch [1024] int64 (all zeros =>
one graph per slab).  Output = x - x[nn_idx] where nn_idx is the argmin
of masked pairwise squared distances per slab.

Strategy (data-parallel over B across 8 cores, 8 slabs per core):
  For each slab:
    NegD[i, j] = x_i . x_j - 0.5*||x_j||^2      (argmax_j NegD = argmin_j d2)
  The self column j == i is always the strict top-1 of each row (d2 == 0),
  so no diagonal mask is needed: take the SECOND entry of the DVE top-8
  (max / max_index) as the nearest neighbor.  Gather x[nn] with an
  indirect DMA and subtract.

Precision: the gram is computed as bf16 hi/lo split (hh + hl + lh three
matmuls, absolute error ~2e-4 on G — fp32-level), and the bias row is a
three-way bf16 split (h/m/l) applied via K=3 matmuls per 128-column block.

Engines:
  PE    : transposes, gram matmuls (bf16 x3) + bias matmuls.
  ACT   : psum->sbuf copies, squares (+row-sum accum), bf16 split rounds.
  DVE   : max (top-8 values) + max_index per 128-row block.
  GPSIMD: split residuals, index offset add, indirect gather, subtract.
"""

import numpy as np

import concourse.bass as bass
import concourse.mybir as mybir
import concourse.tile as tile
from concourse.masks import make_identity

P = 128          # partitions / row-block size
D = 128          # feature dim
NN = 1024        # nodes per slab
NSLAB = 8        # slabs (batch elements) per core
NCORES = 8
NBLK = NN // P   # 8 row blocks per slab

F32 = mybir.dt.float32
BF16 = mybir.dt.bfloat16
U32 = mybir.dt.uint32

ACTF = mybir.ActivationFunctionType
ALU = mybir.AluOpType


def build_nc(split_waits=True):
    nc = bass.Bass()
    x_in = nc.declare_dram_parameter("x", [NSLAB * NN, D], F32, isOutput=False)
    out_ext = nc.declare_dram_parameter("out", [NSLAB * NN, D], F32, isOutput=True)

    with tile.TileContext(nc) as tc:
        with (
            tc.tile_pool(name="const", bufs=1) as cpool,
            tc.tile_pool(name="xhl", bufs=2) as xhlpool,
            tc.tile_pool(name="row", bufs=2) as rowpool,
            tc.tile_pool(name="small", bufs=4) as small,
            tc.tile_pool(name="io", bufs=8) as io,
            tc.tile_pool(name="tp", bufs=2, space="PSUM") as tppool,
            tc.tile_pool(name="tps", bufs=1, space="PSUM") as tpspool,
            tc.tile_pool(name="negd", bufs=2, space="PSUM") as ndpool,
        ):
            ident = cpool.tile([P, P], F32)
            make_identity(nc, ident[:])
            ones12 = cpool.tile([12, P], BF16)    # K=12 lhsT for bias matmul
            nc.gpsimd.memset(ones12[:], 1.0)

            def emit_prep(s):
                base = s * NN
                # ---- transpose slab -> xT (fp32, psum-roundtrip), and sq ----
                xT = rowpool.tile([P, NN], F32, tag="xT")
                sqcol = small.tile([P, NBLK], F32, tag="sqcol")
                for b in range(NBLK):
                    xi = io.tile([P, D], F32, tag="xin")
                    nc.sync.dma_start(xi[:], x_in[base + b * P: base + (b + 1) * P, :])
                    tp = tppool.tile([P, P], F32)
                    nc.tensor.transpose(tp[:], xi[:], ident[:])
                    nc.scalar.activation(xT[:, b * P:(b + 1) * P], tp[:], ACTF.Copy)
                    # sq for rows of this block, as a column
                    sqsc = small.tile([P, D], F32, tag="sqscratch")
                    nc.scalar.activation(
                        sqsc[:], xi[:], ACTF.Square,
                        accum_out=sqcol[:, b:b + 1],
                    )

                # hi/lo bf16 split of xT
                xh = xhlpool.tile([P, NN], BF16, tag="xh")
                nc.scalar.activation(xh[:], xT[:], ACTF.Copy)
                xl = xhlpool.tile([P, NN], BF16, tag="xl")
                nc.gpsimd.tensor_tensor(xl[:], xT[:], xh[:], op=ALU.subtract)

                # ---- bias row: bf16 h/m/l splits of -0.5*sq per block ----
                tps = tpspool.tile([NBLK, P], F32)
                nc.tensor.transpose(tps[:], sqcol[:], ident[:])
                sqr = small.tile([NBLK, P], F32, tag="sqr")
                nc.scalar.activation(sqr[:], tps[:], ACTF.Copy, scale=-0.5)
                hml = []
                prev = sqr
                for t in range(3):
                    st = small.tile([NBLK, P], BF16, tag=f"split{t}")
                    nc.scalar.activation(st[:], prev[:], ACTF.Copy)
                    hml.append(st)
                    if t < 2:
                        r = small.tile([NBLK, P], F32, tag=f"resid{t}")
                        nc.vector.scalar_tensor_tensor(
                            r[:], prev[:], 0.0, st[:], op0=ALU.add, op1=ALU.subtract
                        )
                        prev = r

                # zero-padded bias rhs: rhs_b[t*4+k', jb*512+k'*128+n]
                #   = split_t(-0.5*sq)[block 4*jb+k', n]
                rhs_b = rowpool.tile([12, NN], BF16, tag="rhsb")
                nc.gpsimd.memset(rhs_b[:], 0.0)
                for jb in range(2):
                    for kq in range(4):
                        for t in range(3):
                            eng = nc.sync if (kq + t) % 2 == 0 else nc.scalar
                            eng.dma_start(
                                rhs_b[t * 4 + kq: t * 4 + kq + 1,
                                      jb * 512 + kq * P: jb * 512 + (kq + 1) * P],
                                hml[t][4 * jb + kq: 4 * jb + kq + 1, :],
                            )

                base_col = small.tile([P, 1], U32, tag="basecol")
                nc.gpsimd.memset(base_col[:], base)
                return dict(xh=xh, xl=xl, rhs_b=rhs_b, base_col=base_col)

            def emit_block(s, tl, b):
                base = s * NN
                xh, xl, rhs_b, base_col = tl["xh"], tl["xl"], tl["rhs_b"], tl["base_col"]
                bsl = slice(b * P, (b + 1) * P)
                negd = ndpool.tile([P, NN], F32)
                sls = [slice(0, 512), slice(512, 1024)]
                for jb in range(2):
                    nc.tensor.matmul(
                        negd[:, sls[jb]], lhsT=xh[:, bsl], rhs=xh[:, sls[jb]],
                        start=True, stop=False,
                    )
                for jb in range(2):
                    nc.tensor.matmul(
                        negd[:, sls[jb]], lhsT=xh[:, bsl], rhs=xl[:, sls[jb]],
                        start=False, stop=False,
                    )
                for jb in range(2):
                    nc.tensor.matmul(
                        negd[:, sls[jb]], lhsT=xl[:, bsl], rhs=xh[:, sls[jb]],
                        start=False, stop=False,
                    )
                for jb in range(2):
                    nc.tensor.matmul(
                        negd[:, sls[jb]], lhsT=ones12[:], rhs=rhs_b[:, sls[jb]],
                        start=False, stop=True,
                    )

                v8 = small.tile([P, 8], F32, tag="v8")
                nc.vector.max(v8[:], negd[:])
                i8 = small.tile([P, 8], U32, tag="i8")
                nc.vector.max_index(i8[:], v8[:], negd[:])

                # second-best = nearest neighbor (top-1 is always self)
                idxg = small.tile([P, 1], U32, tag="idxg")
                nc.gpsimd.tensor_tensor(
                    idxg[:], i8[:, 1:2], base_col[:], op=ALU.add
                )

                xnn = io.tile([P, D], F32, tag="xnn")
                nc.gpsimd.indirect_dma_start(
                    out=xnn[:],
                    out_offset=None,
                    in_=x_in[:],
                    in_offset=bass.IndirectOffsetOnAxis(ap=idxg[:, :1], axis=0),
                )

                xi2 = io.tile([P, D], F32, tag="xi2")
                nc.scalar.dma_start(xi2[:], x_in[base + b * P: base + (b + 1) * P, :])
                o = io.tile([P, D], F32, tag="o")
                nc.gpsimd.tensor_tensor(o[:], xi2[:], xnn[:], op=ALU.subtract)
                nc.sync.dma_start(out_ext[base + b * P: base + (b + 1) * P, :], o[:])

            # software-pipelined emission: inject slab s+1 prep after
            # slab s's second block so PE never drains at slab boundaries
            tl = emit_prep(0)
            for s in range(NSLAB):
                nxt = None
                for b in range(NBLK):
                    emit_block(s, tl, b)
                    if b == 1 and s + 1 < NSLAB:
                        nxt = emit_prep(s + 1)
                tl = nxt

    if split_waits:
        _split_excess_waits(nc)
    return nc


# walrus codegen wait-count limits per instruction struct
_WAIT_LIMITS = ((mybir.InstMatmult, 1), (mybir.InstDMACopy, 1))


def _split_excess_waits(nc):
    """walrus codegen allows limited sync waits per instruction (e.g. one on
    a Matmult S3_LW struct); peel extra waits onto preceding same-engine
    NoOps."""
    for fn in nc.m.functions:
        for bb in fn.blocks:
            out = []
            for ins in bb.instructions:
                si = ins.sync_info
                limit = 1
                if si is not None and len(si.on_wait) > limit:
                    for k, w in enumerate(si.on_wait[:-limit]):
                        nop = mybir.InstNoOp(
                            name=f"{ins.name}-w{k}",
                            engine=ins.engine,
                            bass_nofuse=True,
                        )
                        nop.sync_info = mybir.SyncInfo(on_wait=[w], on_update=[])
                        out.append(nop)
                    ins.sync_info = mybir.SyncInfo(
                        on_wait=list(si.on_wait[-limit:]), on_update=si.on_update
                    )
                out.append(ins)
            bb.instructions[:] = out


_NC_CACHE = None
LAST_EXEC_NS = None


def _get_nc():
    global _NC_CACHE
    if _NC_CACHE is None:
        _NC_CACHE = build_nc()
    return _NC_CACHE


def _numpy_fallback(x, batch):
    # General (non-uniform batch) path; mirrors the reference exactly.
    x = np.asarray(x, dtype=np.float32)
    batch = np.asarray(batch)
    B, Nn, d = x.shape
    sq = np.sum(x * x, axis=-1)
    out = np.empty_like(x)
    same = batch[:, None] == batch[None, :]
    np.fill_diagonal(same, False)
    big = np.finfo(np.float32).max
    for b in range(B):
        d2 = sq[b][:, None] + sq[b][None, :] - 2.0 * (x[b] @ x[b].T)
        d2 = np.where(same, d2, big)
        nn_idx = np.argmin(d2, axis=-1)
        out[b] = x[b] - x[b][nn_idx]
    return out


def _ensure_ntff_hook():
    """The agent image's antenv lacks axon_hooks; synthesize it and register
    the ctypes NTFF profile hook so run_bass_kernel_spmd(trace=True) works."""
    import sys
    import types

    if "antenv.axon_hooks" in sys.modules:
        return
    try:
        from trn_agent_boot.trn_boot import _ntff_profile_via_ctypes
        hook = _ntff_profile_via_ctypes("/opt/axon/libaxon_pjrt.so")
    except Exception:
        hook = None
    mod = types.ModuleType("antenv.axon_hooks")
    mod._hook = hook

    def set_axon_ntff_profile_hook(h):
        mod._hook = h

    def get_axon_ntff_profile_hook():
        return mod._hook

    mod.set_axon_ntff_profile_hook = set_axon_ntff_profile_hook
    mod.get_axon_ntff_profile_hook = get_axon_ntff_profile_hook
    sys.modules["antenv.axon_hooks"] = mod


def kernel(x, batch):
    global LAST_EXEC_NS
    x = np.ascontiguousarray(np.asarray(x, dtype=np.float32))
    batch = np.asarray(batch)
    assert x.shape == (NCORES * NSLAB, NN, D), x.shape

    if np.unique(batch).size > 1:
        # batch grouping other than "one graph per slab" — host fallback
        return _numpy_fallback(x, batch)

    from concourse.bass_utils import run_bass_kernel_spmd

    nc = _get_nc()
    in_maps = [
        {"x": x[c * NSLAB:(c + 1) * NSLAB].reshape(NSLAB * NN, D)}
        for c in range(NCORES)
    ]
    import os
    trace = os.environ.get("KERNEL_TRACE", "1") == "1"
    if trace:
        _ensure_ntff_hook()
    res = run_bass_kernel_spmd(
        nc, in_maps, core_ids=list(range(NCORES)), trace=trace,
        tmpdir=os.environ.get("KERNEL_TRACE_DIR") or None,
    )
    LAST_EXEC_NS = res.exec_time_ns
    outs = [res.results[c]["out"].reshape(NSLAB, NN, D) for c in range(NCORES)]
    return np.concatenate(outs, axis=0)


# revision 21
# speedup vs baseline: 1.3931x; 1.0990x over previous
"""MinDist (k=1 nearest neighbor within batch slab) Trainium2 kernel.

Problem: x [B=64, Nn=1024, d=128] f32, batch [1024] int64 (all zeros =>
one graph per slab).  Output = x - x[nn_idx] where nn_idx is the argmin
of masked pairwise squared distances per slab.

Strategy (data-parallel over B across 8 cores, 8 slabs per core):
  For each slab:
    NegD[i, j] = x_i . x_j - 0.5*||x_j||^2      (argmax_j NegD = argmin_j d2)
  The self column j == i is always the strict top-1 of each row (d2 == 0),
  so no diagonal mask is needed: take the SECOND entry of the DVE top-8
  (max / max_index) as the nearest neighbor.  Gather x[nn] with an
  indirect DMA and subtract.

Precision: the gram is computed as bf16 hi/lo split (hh + hl + lh three
matmuls, absolute error ~2e-4 on G — fp32-level), and the bias row is a
three-way bf16 split (h/m/l) applied via K=3 matmuls per 128-column block.

Engines:
  PE    : transposes, gram matmuls (bf16 x3) + bias matmuls.
  ACT   : psum->sbuf copies, squares (+row-sum accum), bf16 split rounds.
  DVE   : max (top-8 values) + max_index per 128-row block.
  GPSIMD: split residuals, index offset add, indirect gather, subtract.
"""

import numpy as np

import concourse.bass as bass
import concourse.mybir as mybir
import concourse.tile as tile
from concourse.masks import make_identity

P = 128          # partitions / row-block size
D = 128          # feature dim
NN = 1024        # nodes per slab
NSLAB = 8        # slabs (batch elements) per core
NCORES = 8
NBLK = NN // P   # 8 row blocks per slab

F32 = mybir.dt.float32
BF16 = mybir.dt.bfloat16
U32 = mybir.dt.uint32

ACTF = mybir.ActivationFunctionType
ALU = mybir.AluOpType


def build_nc(split_waits=True):
    nc = bass.Bass()
    x_in = nc.declare_dram_parameter("x", [NSLAB * NN, D], F32, isOutput=False)
    out_ext = nc.declare_dram_parameter("out", [NSLAB * NN, D], F32, isOutput=True)

    with tile.TileContext(nc) as tc:
        with (
            tc.tile_pool(name="const", bufs=1) as cpool,
            tc.tile_pool(name="xhl", bufs=2) as xhlpool,
            tc.tile_pool(name="row", bufs=2) as rowpool,
            tc.tile_pool(name="small", bufs=4) as small,
            tc.tile_pool(name="io", bufs=8) as io,
            tc.tile_pool(name="tp", bufs=1, space="PSUM") as tppool,
            tc.tile_pool(name="tps", bufs=1, space="PSUM") as tpspool,
            tc.tile_pool(name="negd", bufs=3, space="PSUM") as ndpool,
        ):
            ident = cpool.tile([P, P], F32)
            make_identity(nc, ident[:])
            ones12 = cpool.tile([12, P], BF16)    # K=12 lhsT for bias matmul
            nc.gpsimd.memset(ones12[:], 1.0)

            def emit_prep(s):
                base = s * NN
                # ---- transpose slab -> xT (fp32, psum-roundtrip), and sq ----
                xT = rowpool.tile([P, NN], F32, tag="xT")
                sqcol = small.tile([P, NBLK], F32, tag="sqcol")
                for b in range(NBLK):
                    xi = io.tile([P, D], F32, tag="xin")
                    nc.sync.dma_start(xi[:], x_in[base + b * P: base + (b + 1) * P, :])
                    tp = tppool.tile([P, P], F32)
                    nc.tensor.transpose(tp[:], xi[:], ident[:])
                    nc.scalar.activation(xT[:, b * P:(b + 1) * P], tp[:], ACTF.Copy)
                    # sq for rows of this block, as a column
                    sqsc = small.tile([P, D], F32, tag="sqscratch")
                    nc.scalar.activation(
                        sqsc[:], xi[:], ACTF.Square,
                        accum_out=sqcol[:, b:b + 1],
                    )

                # hi/lo bf16 split of xT
                xh = xhlpool.tile([P, NN], BF16, tag="xh")
                nc.scalar.activation(xh[:], xT[:], ACTF.Copy)
                xl = xhlpool.tile([P, NN], BF16, tag="xl")
                nc.gpsimd.tensor_tensor(xl[:], xT[:], xh[:], op=ALU.subtract)

                # ---- bias row: bf16 h/m/l splits of -0.5*sq per block ----
                tps = tpspool.tile([NBLK, P], F32)
                nc.tensor.transpose(tps[:], sqcol[:], ident[:])
                sqr = small.tile([NBLK, P], F32, tag="sqr")
                nc.scalar.activation(sqr[:], tps[:], ACTF.Copy, scale=-0.5)
                hml = []
                prev = sqr
                for t in range(3):
                    st = small.tile([NBLK, P], BF16, tag=f"split{t}")
                    nc.scalar.activation(st[:], prev[:], ACTF.Copy)
                    hml.append(st)
                    if t < 2:
                        r = small.tile([NBLK, P], F32, tag=f"resid{t}")
                        nc.vector.scalar_tensor_tensor(
                            r[:], prev[:], 0.0, st[:], op0=ALU.add, op1=ALU.subtract
                        )
                        prev = r

                # zero-padded bias rhs: rhs_b[t*4+k', jb*512+k'*128+n]
                #   = split_t(-0.5*sq)[block 4*jb+k', n]
                rhs_b = rowpool.tile([12, NN], BF16, tag="rhsb")
                nc.gpsimd.memset(rhs_b[:], 0.0)
                for jb in range(2):
                    for kq in range(4):
                        for t in range(3):
                            eng = nc.sync if (kq + t) % 2 == 0 else nc.scalar
                            eng.dma_start(
                                rhs_b[t * 4 + kq: t * 4 + kq + 1,
                                      jb * 512 + kq * P: jb * 512 + (kq + 1) * P],
                                hml[t][4 * jb + kq: 4 * jb + kq + 1, :],
                            )

                base_col = small.tile([P, 1], U32, tag="basecol")
                nc.gpsimd.memset(base_col[:], base)
                return dict(xh=xh, xl=xl, rhs_b=rhs_b, base_col=base_col)

            def emit_block(s, tl, b):
                base = s * NN
                xh, xl, rhs_b, base_col = tl["xh"], tl["xl"], tl["rhs_b"], tl["base_col"]
                bsl = slice(b * P, (b + 1) * P)
                negd = ndpool.tile([P, NN], F32)
                sls = [slice(0, 512), slice(512, 1024)]
                for jb in range(2):
                    nc.tensor.matmul(
                        negd[:, sls[jb]], lhsT=xh[:, bsl], rhs=xh[:, sls[jb]],
                        start=True, stop=False,
                    )
                for jb in range(2):
                    nc.tensor.matmul(
                        negd[:, sls[jb]], lhsT=xh[:, bsl], rhs=xl[:, sls[jb]],
                        start=False, stop=False,
                    )
                for jb in range(2):
                    nc.tensor.matmul(
                        negd[:, sls[jb]], lhsT=xl[:, bsl], rhs=xh[:, sls[jb]],
                        start=False, stop=False,
                    )
                for jb in range(2):
                    nc.tensor.matmul(
                        negd[:, sls[jb]], lhsT=ones12[:], rhs=rhs_b[:, sls[jb]],
                        start=False, stop=True,
                    )

                v8 = small.tile([P, 8], F32, tag="v8")
                nc.vector.max(v8[:], negd[:])
                i8 = small.tile([P, 8], U32, tag="i8")
                nc.vector.max_index(i8[:], v8[:], negd[:])

                # second-best = nearest neighbor (top-1 is always self)
                idxg = small.tile([P, 1], U32, tag="idxg")
                nc.gpsimd.tensor_tensor(
                    idxg[:], i8[:, 1:2], base_col[:], op=ALU.add
                )

                xnn = io.tile([P, D], F32, tag="xnn")
                nc.gpsimd.indirect_dma_start(
                    out=xnn[:],
                    out_offset=None,
                    in_=x_in[:],
                    in_offset=bass.IndirectOffsetOnAxis(ap=idxg[:, :1], axis=0),
                )

                xi2 = io.tile([P, D], F32, tag="xi2")
                nc.scalar.dma_start(xi2[:], x_in[base + b * P: base + (b + 1) * P, :])
                o = io.tile([P, D], F32, tag="o")
                nc.gpsimd.tensor_tensor(o[:], xi2[:], xnn[:], op=ALU.subtract)
                nc.sync.dma_start(out_ext[base + b * P: base + (b + 1) * P, :], o[:])

            # software-pipelined emission: inject slab s+1 prep after
            # slab s's second block so PE never drains at slab boundaries
            tl = emit_prep(0)
            for s in range(NSLAB):
                nxt = None
                for b in range(NBLK):
                    emit_block(s, tl, b)
                    if b == 1 and s + 1 < NSLAB:
                        nxt = emit_prep(s + 1)
                tl = nxt

    if split_waits:
        _split_excess_waits(nc)
    return nc


# walrus codegen wait-count limits per instruction struct
_WAIT_LIMITS = ((mybir.InstMatmult, 1), (mybir.InstDMACopy, 1))


def _split_excess_waits(nc):
    """walrus codegen allows limited sync waits per instruction (e.g. one on
    a Matmult S3_LW struct); peel extra waits onto preceding same-engine
    NoOps."""
    for fn in nc.m.functions:
        for bb in fn.blocks:
            out = []
            for ins in bb.instructions:
                si = ins.sync_info
                limit = 1
                if si is not None and len(si.on_wait) > limit:
                    for k, w in enumerate(si.on_wait[:-limit]):
                        nop = mybir.InstNoOp(
                            name=f"{ins.name}-w{k}",
                            engine=ins.engine,
                            bass_nofuse=True,
                        )
                        nop.sync_info = mybir.SyncInfo(on_wait=[w], on_update=[])
                        out.append(nop)
                    ins.sync_info = mybir.SyncInfo(
                        on_wait=list(si.on_wait[-limit:]), on_update=si.on_update
                    )
                out.append(ins)
            bb.instructions[:] = out


_NC_CACHE = None
LAST_EXEC_NS = None


def _get_nc():
    global _NC_CACHE
    if _NC_CACHE is None:
        _NC_CACHE = build_nc()
    return _NC_CACHE


def _numpy_fallback(x, batch):
    # General (non-uniform batch) path; mirrors the reference exactly.
    x = np.asarray(x, dtype=np.float32)
    batch = np.asarray(batch)
    B, Nn, d = x.shape
    sq = np.sum(x * x, axis=-1)
    out = np.empty_like(x)
    same = batch[:, None] == batch[None, :]
    np.fill_diagonal(same, False)
    big = np.finfo(np.float32).max
    for b in range(B):
        d2 = sq[b][:, None] + sq[b][None, :] - 2.0 * (x[b] @ x[b].T)
        d2 = np.where(same, d2, big)
        nn_idx = np.argmin(d2, axis=-1)
        out[b] = x[b] - x[b][nn_idx]
    return out


def _ensure_ntff_hook():
    """The agent image's antenv lacks axon_hooks; synthesize it and register
    the ctypes NTFF profile hook so run_bass_kernel_spmd(trace=True) works."""
    import sys
    import types

    if "antenv.axon_hooks" in sys.modules:
        return
    try:
        from trn_agent_boot.trn_boot import _ntff_profile_via_ctypes
        hook = _ntff_profile_via_ctypes("/opt/axon/libaxon_pjrt.so")
    except Exception:
        hook = None
    mod = types.ModuleType("antenv.axon_hooks")
    mod._hook = hook

    def set_axon_ntff_profile_hook(h):
        mod._hook = h

    def get_axon_ntff_profile_hook():
        return mod._hook

    mod.set_axon_ntff_profile_hook = set_axon_ntff_profile_hook
    mod.get_axon_ntff_profile_hook = get_axon_ntff_profile_hook
    sys.modules["antenv.axon_hooks"] = mod


def kernel(x, batch):
    global LAST_EXEC_NS
    x = np.ascontiguousarray(np.asarray(x, dtype=np.float32))
    batch = np.asarray(batch)
    assert x.shape == (NCORES * NSLAB, NN, D), x.shape

    if np.unique(batch).size > 1:
        # batch grouping other than "one graph per slab" — host fallback
        return _numpy_fallback(x, batch)

    from concourse.bass_utils import run_bass_kernel_spmd

    nc = _get_nc()
    in_maps = [
        {"x": x[c * NSLAB:(c + 1) * NSLAB].reshape(NSLAB * NN, D)}
        for c in range(NCORES)
    ]
    import os
    trace = os.environ.get("KERNEL_TRACE", "1") == "1"
    if trace:
        _ensure_ntff_hook()
    res = run_bass_kernel_spmd(
        nc, in_maps, core_ids=list(range(NCORES)), trace=trace,
        tmpdir=os.environ.get("KERNEL_TRACE_DIR") or None,
    )
    LAST_EXEC_NS = res.exec_time_ns
    outs = [res.results[c]["out"].reshape(NSLAB, NN, D) for c in range(NCORES)]
    return np.concatenate(outs, axis=0)
